# revision 6
# baseline (speedup 1.0000x reference)
"""Trainium2 Bass kernel for nn_Attention_49082886259369.

Computes, per batch b (one batch per NeuronCore, 8 cores data-parallel):
    fac  = tanh(k @ W + q @ U)            [S, D]
    s    = v^T @ fac                      [D, D]
    attn = softmax(s, axis=batch)         <- couples cores: AllReduce max + sum
    out  = v @ attn                       [S, D]

Precision strategy: all three matmuls run single-pass float32r (PE reads
fp32, truncates to fp22/e8m13, multiplies at full bf16 rate when the
moving dim >= 256).  Per-product relative error ~2^-14; accumulated over
the S=2048 contraction this gives |ds| ~ 3e-3 absolute on s (sigma(s)~35),
i.e. ~0.3% relative output error vs the 2e-2 gate -- and 2.5x fewer PE
passes than the 3-pass bf16 hi/lo split (4 units vs 10).

Structure per core:
  sweep (mi = 0..15 row-tiles of S):
    mm1: fac[mi] = tanh(k[mi] @ W + q[mi] @ U), W/U SBUF-resident,
         both 512-wide e-halves per mi (one k/q pass over HBM).
    mm2: every QUAD mi, s[di,ni] += v[mi..mi+3]^T @ fac[mi..mi+3],
         accumulated in PSUM across the quad, one DVE add per quad.
  softmax: 4 e-chunks of 256: AllReduce-max (bf16) -> exp(s-m) ->
         AllReduce-sum (fp32) -> attn = e/den, pipelined.
  mm3: per e-chunk, out[:, chunk] = v @ attn[:, chunk], vT SBUF-resident
       (sweep pools released first), overlapping later chunks' AR-sum.

Layouts pre-tiled on host so every DMA is [128 part x contiguous free]:
  kT/qT/vT: [MT, 128, DT, 128] with [mi,p,di,sj] = x[mi*128+sj, di*128+p]
  W/U:      [DT, 128, D]       with [di,p,e]     = W[di*128+p, e]
  v:        [MT, 128, D]       with [mi,p,d]     = v[mi*128+p, d]
"""

import os

import numpy as np

B, S, D = 8, 2048, 1024
NCORES = 8
P = 128
NE = 512   # e-half width (max fp32 moving operand / one PSUM bank)
NC = 256   # softmax/mm3 chunk width
QUAD = 4   # mi-group size for PSUM accumulation of s

_CACHE: dict = {}


# --------------------------------------------------------------------------
# device kernel builder
# --------------------------------------------------------------------------

def _build_nc(s_dim: int, d_dim: int, n_reps: int = 1, variant: str = "full"):
    import concourse.mybir as mybir
    import concourse.tile as tile
    from concourse import bacc

    F32 = mybir.dt.float32
    F32R = mybir.dt.float32r
    BF16 = mybir.dt.bfloat16
    ACT = mybir.ActivationFunctionType

    MT = s_dim // P          # row tiles of S
    DT = d_dim // P          # row tiles of D
    NH = d_dim // NE         # e-halves
    NCH = d_dim // NC        # softmax chunks

    nc = bacc.Bacc("TRN2", target_bir_lowering=False, num_devices=NCORES)

    d_kT = nc.dram_tensor("kT", [MT, P, DT, P], F32R, kind="ExternalInput")
    d_qT = nc.dram_tensor("qT", [MT, P, DT, P], F32R, kind="ExternalInput")
    d_W = nc.dram_tensor("W", [DT, P, d_dim], F32R, kind="ExternalInput")
    d_U = nc.dram_tensor("U", [DT, P, d_dim], F32R, kind="ExternalInput")
    d_v = nc.dram_tensor("v", [MT, P, d_dim], F32R, kind="ExternalInput")
    d_vT = nc.dram_tensor("vT", [MT, P, DT, P], F32R, kind="ExternalInput")
    d_out = nc.dram_tensor("out", [s_dim, d_dim], F32, kind="ExternalOutput")

    with tile.TileContext(nc) as tc:
        with (
            tc.tile_pool(name="spers", bufs=1) as s_pool,
            tc.tile_pool(name="stat", bufs=2) as stat_pool,
            tc.tile_pool(name="ost", bufs=2) as out_pool,
            tc.tile_pool(name="fps", bufs=2, space="PSUM") as fac_psum,
            tc.tile_pool(name="sps", bufs=2, space="PSUM") as s_psum,
            tc.tile_pool(name="ops", bufs=2, space="PSUM") as out_psum,
            tc.tile_pool(name="dram", bufs=4, space="DRAM") as dram_pool,
        ):
          for _rep in range(n_reps):
            # s accumulator per e-half; also becomes attn in place (f32r so
            # mm3 runs at full rate)
            s_half = [
                s_pool.tile([P, DT, NE], F32R, tag=f"s_sb{ni}",
                            name=f"s_sb{ni}")
                for ni in range(NH)
            ]

            # ---------------- sweep: mm1 + mm2 ----------------
            with (
                tc.tile_pool(name="wu", bufs=1) as wu_pool,
                tc.tile_pool(name="kq", bufs=2) as kq_pool,
                tc.tile_pool(name="vv", bufs=QUAD + 2) as v_pool,
                tc.tile_pool(name="fac", bufs=QUAD + 2) as fac_pool,
            ):
                # prefetch mi=0 operands ahead of the bulky W/U loads
                kt0 = kq_pool.tile([P, DT, P], F32R, tag="kt", name="kt0")
                qt0 = kq_pool.tile([P, DT, P], F32R, tag="qt", name="qt0")
                v0 = v_pool.tile([P, d_dim], F32R, tag="v", name="v0")
                nc.sync.dma_start(out=kt0, in_=d_kT[0])
                nc.sync.dma_start(out=qt0, in_=d_qT[0])
                nc.sync.dma_start(out=v0, in_=d_v[0])
                w_sb, u_sb = [], []
                for di in range(DT):
                    wt = wu_pool.tile([P, d_dim], F32R, tag=f"w{di}",
                                      name=f"w{di}")
                    ut = wu_pool.tile([P, d_dim], F32R, tag=f"u{di}",
                                      name=f"u{di}")
                    nc.sync.dma_start(out=wt, in_=d_W[di])
                    nc.sync.dma_start(out=ut, in_=d_U[di])
                    w_sb.append(wt)
                    u_sb.append(ut)

                quad_facs = [[], []]
                quad_vs = []
                for mi in range(MT):
                    if mi == 0:
                        kt, qt, vt = kt0, qt0, v0
                    else:
                        kt = kq_pool.tile([P, DT, P], F32R, tag="kt")
                        qt = kq_pool.tile([P, DT, P], F32R, tag="qt")
                        vt = v_pool.tile([P, d_dim], F32R, tag="v")
                        nc.sync.dma_start(out=kt, in_=d_kT[mi])
                        nc.sync.dma_start(out=qt, in_=d_qT[mi])
                        nc.sync.dma_start(out=vt, in_=d_v[mi])

                    # mm1: fac[mi] = k@W + q@U, both halves; one stationary
                    # load serves both halves' moving passes
                    fps = [
                        fac_psum.tile([P, NE], F32, tag=f"fps{ni}",
                                      name=f"fps{ni}")
                        for ni in range(NH)
                    ]
                    for di in range(DT):
                        nc.tensor.matmul(fps[0], kt[:, di, :],
                                         w_sb[di][:, 0:NE],
                                         start=(di == 0), stop=False)
                        nc.tensor.matmul(fps[1], kt[:, di, :],
                                         w_sb[di][:, NE:d_dim],
                                         start=(di == 0), stop=False)
                    for di in range(DT):
                        last = di == DT - 1
                        nc.tensor.matmul(fps[0], qt[:, di, :],
                                         u_sb[di][:, 0:NE],
                                         start=False, stop=last)
                        nc.tensor.matmul(fps[1], qt[:, di, :],
                                         u_sb[di][:, NE:d_dim],
                                         start=False, stop=last)
                    for ni in range(NH):
                        fc = fac_pool.tile([P, NE], F32R, tag=f"fac{ni}",
                                           name=f"fac{ni}")
                        nc.scalar.activation(fc, fps[ni], ACT.Tanh)
                        quad_facs[ni].append(fc)
                    quad_vs.append(vt)

                    # mm2: every QUAD mi, accumulate in PSUM then one DVE add
                    if mi % QUAD == QUAD - 1:
                        first_quad = mi < QUAD
                        for ni in range(NH):
                            for di in range(DT):
                                sps = s_psum.tile([P, NE], F32, tag="sps")
                                for j in range(QUAD):
                                    nc.tensor.matmul(
                                        sps,
                                        quad_vs[j][:, di * P:(di + 1) * P],
                                        quad_facs[ni][j],
                                        start=(j == 0),
                                        stop=(j == QUAD - 1),
                                    )
                                dst = s_half[ni][:, di, :]
                                if first_quad:
                                    nc.vector.tensor_copy(dst, sps)
                                else:
                                    nc.vector.tensor_add(dst, dst, sps)
                        quad_facs = [[], []]
                        quad_vs = []

            if variant == "nosm":
                for mi in range(MT):
                    ost = out_pool.tile([P, d_dim], F32, tag="ost2")
                    nc.vector.tensor_copy(ost[:, :NE], s_half[0][:, 0, :])
                    nc.sync.dma_start(out=d_out[mi * P:(mi + 1) * P, :],
                                      in_=ost)
                continue

            # ---------------- softmax: AR-max, exp, AR-sum ----------------
            # all chunks' max collectives first (s is fully ready), then
            # per-chunk exp + AR-sum; collective queue: max0..3, sum0..3
            cc_max_tiles = []
            for ci in range(NCH):
                ni, lsl = ci // (NE // NC), slice((ci % 2) * NC,
                                                  (ci % 2) * NC + NC)
                s_bf = stat_pool.tile([P, DT, NC], BF16, tag="s_bf",
                                      name=f"s_bf{ci}")
                nc.vector.tensor_copy(s_bf, s_half[ni][:, :, lsl])
                cc_s_in = dram_pool.tile([P, DT, NC], BF16, tag="cc_s_in",
                                         name=f"cc_s_in{ci}")
                cc_s_max = dram_pool.tile([P, DT, NC], BF16, tag="cc_s_max",
                                          name=f"cc_s_max{ci}",
                                          addr_space="Shared")
                nc.sync.dma_start(out=cc_s_in, in_=s_bf)
                if variant != "nocc":
                    nc.gpsimd.collective_compute(
                        "AllReduce",
                        mybir.AluOpType.max,
                        replica_groups=[list(range(NCORES))],
                        ins=[cc_s_in.opt()],
                        outs=[cc_s_max.opt()],
                    )
                else:
                    nc.gpsimd.dma_start(out=cc_s_max[:], in_=cc_s_in[:])
                cc_max_tiles.append(cc_s_max)

            cc_sum_tiles = []
            for ci in range(NCH):
                ni, lsl = ci // (NE // NC), slice((ci % 2) * NC,
                                                  (ci % 2) * NC + NC)
                sh = s_half[ni]
                m_sb = stat_pool.tile([P, DT, NC], BF16, tag="m_sb")
                nc.sync.dma_start(out=m_sb, in_=cc_max_tiles[ci])
                for di in range(DT):
                    nc.vector.tensor_sub(
                        sh[:, di, lsl], sh[:, di, lsl], m_sb[:, di, :]
                    )
                    nc.scalar.activation(sh[:, di, lsl], sh[:, di, lsl],
                                         ACT.Exp)
                cc_e_in = dram_pool.tile([P, DT, NC], F32R, tag="cc_e_in",
                                         name=f"cc_e_in{ci}")
                cc_e_sum = dram_pool.tile([P, DT, NC], F32R, tag="cc_e_sum",
                                          name=f"cc_e_sum{ci}",
                                          addr_space="Shared")
                nc.sync.dma_start(out=cc_e_in, in_=sh[:, :, lsl])
                if variant != "nocc":
                    nc.gpsimd.collective_compute(
                        "AllReduce",
                        mybir.AluOpType.add,
                        replica_groups=[list(range(NCORES))],
                        ins=[cc_e_in.opt()],
                        outs=[cc_e_sum.opt()],
                    )
                else:
                    nc.sync.dma_start(out=cc_e_sum[:], in_=cc_e_in[:])
                cc_sum_tiles.append(cc_e_sum)

            # ---------------- attn + mm3, per chunk ----------------
            with tc.tile_pool(name="vt", bufs=1) as vt_pool:
                vtt = [
                    vt_pool.tile([P, DT, P], F32R, tag=f"vtt{mi}",
                                 name=f"vtt{mi}")
                    for mi in range(MT)
                ]
                for mi in range(MT):
                    nc.sync.dma_start(out=vtt[mi], in_=d_vT[mi])
                for ci in range(NCH):
                    ni, lsl = ci // (NE // NC), slice((ci % 2) * NC,
                                                      (ci % 2) * NC + NC)
                    sh = s_half[ni]
                    den = stat_pool.tile([P, DT, NC], F32R, tag="den")
                    nc.sync.dma_start(out=den, in_=cc_sum_tiles[ci])
                    for di in range(DT):
                        with nc.allow_low_precision(
                                reason="f32r bytes are fp32"):
                            nc.vector.reciprocal(den[:, di, :],
                                                 den[:, di, :])
                        nc.vector.tensor_mul(
                            sh[:, di, lsl], sh[:, di, lsl], den[:, di, :]
                        )
                    e0 = ni * NE + lsl.start
                    for mi in range(MT):
                        ops = out_psum.tile([P, NC], F32, tag="ops")
                        for di in range(DT):
                            nc.tensor.matmul(
                                ops,
                                vtt[mi][:, di, :],
                                sh[:, di, lsl],
                                start=(di == 0),
                                stop=(di == DT - 1),
                            )
                        ost = out_pool.tile([P, NC], F32, tag="ost")
                        nc.vector.tensor_copy(ost, ops)
                        nc.sync.dma_start(
                            out=d_out[mi * P:(mi + 1) * P, e0:e0 + NC],
                            in_=ost,
                        )
            tc.tile_update_base_wait()

    nc.compile()
    return nc


def _get_nc(s_dim=S, d_dim=D, n_reps=1, variant="full"):
    key = ("nc", s_dim, d_dim, n_reps, variant)
    if key not in _CACHE:
        _CACHE[key] = _build_nc(s_dim, d_dim, n_reps, variant)
    return _CACHE[key]


# --------------------------------------------------------------------------
# host-side packing
# --------------------------------------------------------------------------

def _tileT(x: np.ndarray, s_dim: int, d_dim: int) -> np.ndarray:
    """[S, D] -> [MT, 128, DT, 128] with [mi,p,di,sj] = x[mi*128+sj, di*128+p]."""
    mt, dt = s_dim // P, d_dim // P
    return np.ascontiguousarray(
        x.reshape(mt, P, dt, P).transpose(0, 3, 2, 1)
    )


def prepare_in_maps(q, k, v, W, U, s_dim=S, d_dim=D):
    q = np.asarray(q, dtype=np.float32)
    k = np.asarray(k, dtype=np.float32)
    v = np.asarray(v, dtype=np.float32)
    W = np.asarray(W, dtype=np.float32)
    U = np.asarray(U, dtype=np.float32)

    dt = d_dim // P
    mt = s_dim // P
    W_t = np.ascontiguousarray(W.reshape(dt, P, d_dim))
    U_t = np.ascontiguousarray(U.reshape(dt, P, d_dim))

    in_maps = []
    for b in range(NCORES):
        in_maps.append({
            "kT": _tileT(k[b], s_dim, d_dim),
            "qT": _tileT(q[b], s_dim, d_dim),
            "vT": _tileT(v[b], s_dim, d_dim),
            "W": W_t, "U": U_t,
            "v": np.ascontiguousarray(v[b].reshape(mt, P, d_dim)),
        })
    return in_maps


def run_spmd(in_maps, s_dim=S, d_dim=D):
    """One-shot path through the stock bass_utils helper (debug use)."""
    from concourse import bass_utils
    nc = _get_nc(s_dim, d_dim)
    res = bass_utils.run_bass_kernel_spmd(
        nc, in_maps=in_maps, core_ids=list(range(NCORES))
    )
    return res


def _get_runner(s_dim=S, d_dim=D, n_reps=1, variant="full"):
    """Cached sharded-jit runner over the same bass2jax/_bass_exec_p path
    that bass_utils.run_bass_kernel_spmd uses under axon, but built once per
    process (no donation) so repeat calls skip re-trace/re-compile."""
    key = ("runner", s_dim, d_dim, n_reps, variant)
    if key in _CACHE:
        return _CACHE[key]

    import jax
    from jax.sharding import Mesh, PartitionSpec
    from jax.experimental.shard_map import shard_map
    import concourse.mybir as mybir
    from concourse import bass2jax

    nc = _get_nc(s_dim, d_dim, n_reps, variant)
    bass2jax.install_neuronx_cc_hook()

    partition_name = (
        nc.partition_id_tensor.name if nc.partition_id_tensor else None
    )
    in_names, out_names, out_avals, zero_outs = [], [], [], []
    for alloc in nc.m.functions[0].allocations:
        if not isinstance(alloc, mybir.MemoryLocationSet):
            continue
        name = alloc.memorylocations[0].name
        if alloc.kind == "ExternalInput":
            if name != partition_name:
                in_names.append(name)
        elif alloc.kind == "ExternalOutput":
            shape = tuple(alloc.tensor_shape)
            dtype = mybir.dt.np(alloc.dtype)
            out_names.append(name)
            out_avals.append(jax.core.ShapedArray(shape, dtype))
            zero_outs.append(np.zeros(shape, dtype))
    n_params = len(in_names)
    all_in_names = list(in_names) + list(out_names)
    if partition_name is not None:
        all_in_names.append(partition_name)

    def _body(*args):
        operands = list(args)
        if partition_name is not None:
            operands.append(bass2jax.partition_id_tensor())
        outs = bass2jax._bass_exec_p.bind(
            *operands,
            out_avals=tuple(out_avals),
            in_names=tuple(all_in_names),
            out_names=tuple(out_names),
            lowering_input_output_aliases=(),
            sim_require_finite=True,
            sim_require_nnan=True,
            nc=nc,
        )
        return tuple(outs)

    devices = jax.devices()[:NCORES]
    mesh = Mesh(np.asarray(devices), ("core",))
    in_specs = (PartitionSpec("core"),) * (n_params + len(out_names))
    out_specs = (PartitionSpec("core"),) * len(out_names)
    sharded = jax.jit(
        shard_map(
            _body, mesh=mesh, in_specs=in_specs, out_specs=out_specs,
            check_rep=False,
        ),
        keep_unused=True,
    )
    runner = {
        "fn": sharded,
        "in_names": in_names,
        "out_names": out_names,
        "out_avals": out_avals,
        "zero_concat": [
            np.zeros((NCORES * z.shape[0], *z.shape[1:]), z.dtype)
            for z in zero_outs
        ],
        "mesh": mesh,
    }
    _CACHE[key] = runner
    return runner


def _concat_inputs(runner, in_maps):
    return [
        np.concatenate([np.asarray(m[name]) for m in in_maps], axis=0)
        for name in runner["in_names"]
    ]


def run_fast(in_maps, s_dim=S, d_dim=D):
    """Execute via the cached runner; returns list of per-core out dicts."""
    runner = _get_runner(s_dim, d_dim)
    concat_in = _concat_inputs(runner, in_maps)
    out_arrs = runner["fn"](*concat_in, *runner["zero_concat"])
    results = []
    for c in range(NCORES):
        results.append({
            name: np.asarray(out_arrs[i]).reshape(
                NCORES, *runner["out_avals"][i].shape
            )[c]
            for i, name in enumerate(runner["out_names"])
        })
    return results


def timed_run(in_maps, iters=20, s_dim=S, d_dim=D, n_reps=1, variant="full"):
    """Steady-state timing with device-resident inputs. Returns (min_s, all)."""
    import time
    import jax
    from jax.sharding import NamedSharding, PartitionSpec

    runner = _get_runner(s_dim, d_dim, n_reps, variant)
    sh = NamedSharding(runner["mesh"], PartitionSpec("core"))
    dev_in = [jax.device_put(a, sh) for a in _concat_inputs(runner, in_maps)]
    dev_zero = [jax.device_put(z, sh) for z in runner["zero_concat"]]
    jax.block_until_ready(dev_in)
    jax.block_until_ready(dev_zero)
    # warmup (also triggers compile on first use)
    jax.block_until_ready(runner["fn"](*dev_in, *dev_zero))
    times = []
    for _ in range(iters):
        t0 = time.perf_counter()
        jax.block_until_ready(runner["fn"](*dev_in, *dev_zero))
        times.append(time.perf_counter() - t0)
    return min(times), times


def kernel(q, k, v, W, U):
    in_maps = prepare_in_maps(q, k, v, W, U)
    if os.environ.get("BASS_USE_SPMD_HELPER"):
        res = run_spmd(in_maps)
        results = res.results
    else:
        results = run_fast(in_maps)
    out = np.stack([results[b]["out"] for b in range(NCORES)], axis=0)
    return out.astype(np.float32)


def timed_slope(in_maps, iters=30, reps_hi=3, s_dim=S, d_dim=D, variant="full"):
    """True HW kernel time via replication slope: the reps_hi variant runs
    the whole kernel body reps_hi times inside one NEFF. Calls of the two
    variants are interleaved in one loop so slow network drift cancels;
    returns (per_rep_seconds from median pairwise delta, t1_min, thi_min)."""
    import time
    import jax
    from jax.sharding import NamedSharding, PartitionSpec

    runners = {}
    for n in (1, reps_hi):
        r = _get_runner(s_dim, d_dim, n, variant)
        sh = NamedSharding(r["mesh"], PartitionSpec("core"))
        dev_in = [jax.device_put(a, sh) for a in _concat_inputs(r, in_maps)]
        dev_zero = [jax.device_put(z, sh) for z in r["zero_concat"]]
        jax.block_until_ready(dev_in)
        jax.block_until_ready(dev_zero)
        jax.block_until_ready(r["fn"](*dev_in, *dev_zero))  # warm/compile
        runners[n] = (r["fn"], dev_in, dev_zero)

    deltas, t1s, this_ = [], [], []
    for _ in range(iters):
        fn, di, dz = runners[1]
        t0 = time.perf_counter()
        jax.block_until_ready(fn(*di, *dz))
        t1 = time.perf_counter() - t0
        fn, di, dz = runners[reps_hi]
        t0 = time.perf_counter()
        jax.block_until_ready(fn(*di, *dz))
        th = time.perf_counter() - t0
        deltas.append(th - t1)
        t1s.append(t1)
        this_.append(th)
    deltas.sort()
    med = deltas[len(deltas) // 2]
    return med / (reps_hi - 1), min(t1s), min(this_)
